# revision 1
# baseline (speedup 1.0000x reference)
"""Trainium2 Bass kernel for BudgetAttentionTwo.

Module: keys = x@Wk.T+bk, values = x@Wv.T+bv (split into 8 heads of 64),
S = K K^T per (b, h), out = (softmax(S)/sqrt(E)) @ V, merged back to [B,N,E].

Sharding: 8 cores, each core owns one batch b = core//2 and four heads
hg*4..hg*4+3 (hg = core%2). No cross-device comms. Weights are pre-sliced
and pre-transposed on the host; each core computes its 4 [N,N] attention
blocks entirely locally.

Device-side layout (per core):
  - x^T tiles feed both projections (contraction over E on partitions).
  - KT2[pair] [128, 2048] holds two heads' keys transposed (2*64 d rows).
  - Scores are computed transposed-symmetric: S^T block [128 k, 512 q] via
    lhsT = KT2[:, k-chunk], rhs = KT2[:, q-range]; since S is symmetric this
    is also the [q, k] block. Even/odd heads sit on PE row-groups 0-63 /
    64-127 (tile_position auto-derived from base partition).
  - P = exp(S - 88) unnormalized (constant shift is exact for softmax; max
    logit is bounded ~119 so no overflow, and underflow is ~e^-21 relative).
  - attV: out^T [65, 512] = sum_k [V|ones]^T @ P-chunk; row 64 = row-sums.
    V is pre-scaled by 1/sqrt(E) with bias folded in, so out = PV'/rowsum.
  - Normalize: broadcast the raw row-sum across partitions via a K=1 matmul,
    then one DVE tensor_tensor divide. (A DVE reciprocal here costs ~6
    cycles/elem and stalls the PE FIFO at the broadcast matmul - avoid.)
    Output stays transposed [64 d, N]; host transposes while gathering.

fp32r (rounded fp32, ~1e-4 matmul rel-err) is used for all matmuls: it runs
at bf16 speed (1 cycle/row) when the moving dim >= 256, vs 4 cycles/row for
plain fp32.

Emission order interleaves the projections with the first q-range's
scores/exp so the ACT engine starts its ~130us of exp work ~15us earlier,
and the V projection runs on PE underneath the first exp batch.
"""
import numpy as np

import concourse.bacc as bacc
import concourse.mybir as mybir
import concourse.tile as tile
from concourse.bass_utils import run_bass_kernel_spmd

F32 = mybir.dt.float32
F32R = mybir.dt.float32r
EXP = mybir.ActivationFunctionType.Exp

B, N, E, H = 4, 2048, 512, 8
D = E // H            # 64
NCORES = 8
HPC = 4               # heads per core
CSHIFT = 88.0         # exp(S - CSHIFT)
QW = 512              # q-range width
NQR = N // QW         # 4
KC = N // 128         # 16 k-chunks
GRP = 3               # score chunks per psum tile (3 banks)

_last_results = None  # stashed BassKernelResults for test.py introspection


def _register_const(nc, val):
    """Extra pre-TileContext f32 [128,1] constant (dep-free, like Bass's
    built-in consts) so activation(bias=val) needs no semaphore wait."""
    t = nc.alloc_sbuf_tensor(f"const-float32-{val}", [128, 1], F32)
    nc.gpsimd.memset(t.ap(), val)
    nc.const_aps.aps[(F32, float(val))] = t.ap()
    nc.all_engine_barrier()


def build_program():
    nc = bacc.Bacc()
    _register_const(nc, -CSHIFT)

    xt = nc.dram_tensor("xt", [E, N], F32R, kind="ExternalInput")
    wkt = nc.dram_tensor("wkt", [E, 2 * 128], F32R, kind="ExternalInput")
    wvt = nc.dram_tensor("wvt", [E, 2 * 128], F32R, kind="ExternalInput")
    bk2 = nc.dram_tensor("bk2", [2, 128, 1], F32, kind="ExternalInput")
    bvb = nc.dram_tensor("bvb", [128, 2 * 128], F32, kind="ExternalInput")
    vinit = nc.dram_tensor("vinit", [128, HPC * (D + 1)], F32R, kind="ExternalInput")
    ones64 = nc.dram_tensor("ones64", [33, D], F32R, kind="ExternalInput")
    zrow = nc.dram_tensor("zrow", [1, N], F32R, kind="ExternalInput")
    out_t = nc.dram_tensor("out_t", [HPC, D, N], F32, kind="ExternalOutput")

    with nc.allow_low_precision(reason="fp32r rounding for PE speed is intentional"), \
         tile.TileContext(nc) as tc:
        with (
            tc.tile_pool(name="persist", bufs=1) as per,
            tc.tile_pool(name="work", bufs=2) as work,
            tc.tile_pool(name="mps", bufs=1, space="PSUM") as mps,
        ):
            # ---- persistent SBUF ----
            kt2 = [per.tile([128, N], F32R, name=f"kt2_{p}") for p in range(2)]
            # block-diagonal rhs copies: bd[0][p] = [KT_even; 0],
            # bd[1][p] = [0; KT_odd]. Scores then contract over K=128 (half
            # zeros) which keeps the PE's activity monitor at full clock and
            # shares one LDWEIGHTS between the pair's two matmuls.
            bd = [[per.tile([128, N], F32R, name=f"bd_{j}_{p}")
                   for p in range(2)] for j in range(2)]
            vs = [per.tile([128, HPC * (D + 1)], F32R, name=f"vs_{t}")
                  for t in range(KC)]
            bvb_sb = per.tile([128, HPC * D], F32)
            bk_sb = [per.tile([128, 1], F32, name=f"bk_{p}") for p in range(2)]
            ones1 = per.tile([33, D], F32R)

            nc.sync.dma_start(out=bvb_sb, in_=bvb[:])
            for p in range(2):
                zb = bass_zero_bcast = zrow[:].partition_broadcast(64)
                nc.sync.dma_start(out=bd[0][p][64:128, :], in_=zb)
                nc.sync.dma_start(out=bd[1][p][0:64, :], in_=zb)
            for p in range(2):
                nc.sync.dma_start(out=bk_sb[p], in_=bk2[p])
            nc.sync.dma_start(out=ones1, in_=ones64[:])
            pacc_n = [0]

            def proj_kt2(p):
                # KT2[p][:, q] = (Wk_pair @ x^T + bk_pair)[:, q], fp32r
                for qr in range(NQR):
                    tg = "av" if pacc_n[0] % 2 == 0 else "bc"
                    pacc_n[0] += 1
                    acc = mps.tile([128, QW], F32, tag=tg, bufs=1,
                                   name=f"kacc_{p}_{qr}")
                    for c in range(4):
                        nc.tensor.matmul(
                            acc[:],
                            wkt_sb[c][:, 128 * p:128 * (p + 1)],
                            xt_sb[c][:, QW * qr:QW * (qr + 1)],
                            start=(c == 0), stop=(c == 3),
                        )
                    qs = slice(QW * qr, QW * (qr + 1))
                    nc.vector.tensor_scalar_add(kt2[p][:, qs], acc[:],
                                                bk_sb[p][:])
                    nc.vector.tensor_scalar_add(bd[0][p][0:64, qs],
                                                acc[0:64, :], bk_sb[p][0:64])
                    nc.vector.tensor_scalar_add(bd[1][p][64:128, qs],
                                                acc[64:128, :],
                                                bk_sb[p][64:128])

            def proj_v():
                # V tiles [128 n, 4 heads * 64] + bias, with a trailing ones
                # column per head: [V_h0|1|V_h1|1|V_h2|1|V_h3|1]
                for t in range(KC):
                    tg = "av" if pacc_n[0] % 2 == 0 else "bc"
                    pacc_n[0] += 1
                    acc = mps.tile([128, QW], F32, tag=tg, bufs=1,
                                   name=f"vacc_{t}")
                    for c in range(4):
                        nc.tensor.matmul(
                            acc[:, :HPC * D],
                            xt_sb[c][:, 128 * t:128 * (t + 1)],
                            wvt_sb[c][:],
                            start=(c == 0), stop=(c == 3),
                        )
                    nc.sync.dma_start(out=vs[t], in_=vinit[:])
                    vst = vs[t].rearrange("p (h y) -> p h y", h=HPC)
                    nc.vector.tensor_tensor(
                        out=vst[:, :, 0:D],
                        in0=acc[:, :HPC * D].rearrange("p (h d) -> p h d", h=HPC),
                        in1=bvb_sb.rearrange("p (h d) -> p h d", h=HPC),
                        op=mybir.AluOpType.add,
                    )

            def scores_exp(p, qr, pts, mid_cb=None):
                q0 = QW * qr
                for g in range(0, KC, GRP):
                    if g == 2 * GRP and mid_cb is not None:
                        mid_cb()
                    w = min(GRP, KC - g)
                    sc = [mps.tile([128, GRP * QW], F32, tag="sc", bufs=2,
                                   name=f"sc_{p}_{qr}_{g}_{j}")
                          for j in range(2)]
                    for i in range(w):
                        kc = g + i
                        for j in range(2):
                            nc.tensor.matmul(
                                sc[j][:, QW * i:QW * (i + 1)],
                                kt2[p][:, 128 * kc:128 * (kc + 1)],
                                bd[j][p][:, q0:q0 + QW],
                                start=True, stop=True,
                            )
                    for j in range(2):
                        nc.scalar.activation(
                            pts[j][:, QW * g:QW * (g + w)],
                            sc[j][:, :QW * w],
                            EXP, bias=-CSHIFT, scale=1.0,
                        )

            def attv_mm(p, qr, pts):
                avs = []
                rb = work.tile([33, QW], F32, tag="rb", bufs=2,
                               name=f"rb_{p}_{qr}")
                for j in range(2):
                    hl = 2 * p + j
                    av = mps.tile([D + 1, QW], F32, tag="av", bufs=1,
                                  name=f"av_{p}_{qr}_{j}")
                    for kc in range(KC):
                        vsl = vs[kc].rearrange("p (h y) -> p h y", h=HPC)
                        nc.tensor.matmul(
                            av[:],
                            vsl[:, hl, :],
                            pts[j][:, QW * kc:QW * (kc + 1)],
                            start=(kc == 0), stop=(kc == KC - 1),
                        )
                    av_sb = work.tile([D + 1, QW], F32, tag="avsb", bufs=3,
                                      name=f"avsb_{p}_{qr}_{j}")
                    nc.vector.tensor_copy(av_sb[:], av[:])
                    # both heads' row-sums into one tile (rows 0 / 32) so a
                    # single batched reciprocal serves the pair
                    nc.vector.tensor_copy(rb[32 * j:32 * j + 1, :],
                                          av_sb[D:D + 1, :])
                    avs.append(av_sb)
                rr = work.tile([33, QW], F32R, tag="rr", bufs=2,
                               name=f"rr_{p}_{qr}")
                nc.vector.reciprocal(rr[:], rb[:])
                return (p, qr, avs, rr)

            def epilogue(state):
                # Emitted two score-groups into the NEXT iteration so the
                # broadcast matmuls never block the PE FIFO on the DVE
                # reciprocal chain. bc-O borrows the "av" psum slot (idle
                # until the next attV, which is an exp-batch away).
                p, qr, avs, rr = state
                q0 = QW * qr
                for j, tg in ((0, "bc"), (1, "av")):
                    hl = 2 * p + j
                    bc = mps.tile([D, QW], F32, tag=tg, bufs=1,
                                  name=f"bc_{p}_{qr}_{j}")
                    nc.tensor.matmul(bc[:], ones1[32 * j:32 * j + 1, :],
                                     rr[32 * j:32 * j + 1, :],
                                     start=True, stop=True)
                    fin = work.tile([D, QW], F32, tag="fin", bufs=2,
                                    name=f"fin_{p}_{qr}_{j}")
                    nc.vector.tensor_tensor(
                        out=fin[:], in0=avs[j][0:D, :], in1=bc[:],
                        op=mybir.AluOpType.mult)
                    nc.sync.dma_start(
                        out=out_t[hl, :, q0:q0 + QW], in_=fin[:])

            # ---- emission: projections (transient x/W tiles), then the
            # main loop with its own big PT pool (stack-allocated after the
            # projection inputs are freed) ----
            with tc.tile_pool(name="pin", bufs=1) as pin:
                xt_sb = [pin.tile([128, N], F32R, name=f"xt_{c}")
                         for c in range(4)]
                wkt_sb = [pin.tile([128, 2 * 128], F32R, name=f"wkt_{c}")
                          for c in range(4)]
                wvt_sb = [pin.tile([128, 2 * 128], F32R, name=f"wvt_{c}")
                          for c in range(4)]
                for c in range(4):
                    nc.sync.dma_start(out=wkt_sb[c],
                                      in_=wkt[128 * c:128 * (c + 1), :])
                    eng = nc.sync if c % 2 == 0 else nc.gpsimd
                    eng.dma_start(out=xt_sb[c], in_=xt[128 * c:128 * (c + 1), :])
                    nc.sync.dma_start(out=wvt_sb[c],
                                      in_=wvt[128 * c:128 * (c + 1), :])
                proj_kt2(0)
                proj_v()
                proj_kt2(1)

            with tc.tile_pool(name="ptp", bufs=1) as ptp:
                pending = None
                for p in range(2):
                    for qr in range(NQR):
                        pts = [ptp.tile([128, KC * QW], F32R, tag="pt", bufs=2,
                                        name=f"pt_{p}_{qr}_{j}")
                               for j in range(2)]
                        st = pending
                        scores_exp(p, qr, pts,
                                   mid_cb=(None if st is None
                                           else (lambda s=st: epilogue(s))))
                        pending = attv_mm(p, qr, pts)
                epilogue(pending)

    nc.finalize()
    return nc


_program = None


def _vinit():
    v = np.zeros((128, HPC * (D + 1)), dtype=np.float32)
    v[:, D::D + 1] = 1.0
    return v


def kernel(x, Wk, bk, Wv, bv):
    global _program, _last_results
    x = np.asarray(x, dtype=np.float32)
    Wk = np.asarray(Wk, dtype=np.float32)
    bk = np.asarray(bk, dtype=np.float32)
    Wv = np.asarray(Wv, dtype=np.float32)
    bv = np.asarray(bv, dtype=np.float32)

    if _program is None:
        _program = build_program()

    sq = np.float32(1.0 / np.sqrt(E))
    in_maps = []
    for c in range(NCORES):
        b, hg = c // 2, c % 2
        cols = slice(hg * HPC * D, (hg + 1) * HPC * D)
        in_maps.append({
            "xt": np.ascontiguousarray(x[b].T),                      # [E, N]
            "wkt": np.ascontiguousarray(Wk[cols, :].T),              # [E, 256]
            "wvt": np.ascontiguousarray(Wv[cols, :].T) * sq,         # [E, 256]
            "bk2": np.ascontiguousarray(bk[cols].reshape(2, 128, 1)),
            "bvb": np.ascontiguousarray(
                np.broadcast_to(bv[cols] * sq, (128, HPC * D))),
            "vinit": _vinit(),
            "ones64": np.ones((33, D), dtype=np.float32),
            "zrow": np.zeros((1, N), dtype=np.float32),
        })

    import os
    trace = bool(int(os.environ.get("KERNEL_PROFILE", "0")))
    res = run_bass_kernel_spmd(_program, in_maps, list(range(NCORES)),
                               trace=trace)
    _last_results = res

    out = np.empty((B, N, E), dtype=np.float32)
    for c in range(NCORES):
        b, hg = c // 2, c % 2
        ot = res.results[c]["out_t"]                                 # [4, 64, N]
        for hl in range(HPC):
            out[b, :, hg * HPC * D + hl * D:(hg * HPC * D) + (hl + 1) * D] = \
                ot[hl].T
    return out



# revision 6
# speedup vs baseline: 1.2705x; 1.2705x over previous
"""Trainium2 Bass kernel for BudgetAttentionTwo.

Module: keys = x@Wk.T+bk, values = x@Wv.T+bv (split into 8 heads of 64),
S = K K^T per (b, h), out = (softmax(S)/sqrt(E)) @ V, merged back to [B,N,E].

Sharding: 8 cores, each core owns one batch b = core//2 and four heads
hg*4..hg*4+3 (hg = core%2). No cross-device comms. Weights are pre-sliced,
pre-transposed, and pre-cast to bf16 on the host.

All matmul operands are bf16 (f32 PSUM accumulation; host numpy emulation
puts end-to-end rel err at ~3.3e-3 vs the 2e-2 gate). bf16 buys:
fast-weight-load on 128-wide stationary operands, half the input DMA, and a
pts footprint small enough to double-buffer across iterations (the
baseline's per-iteration pipeline bubble came from pts tiles that could not
overlap, re-throttling the PE's HAM clock every iteration). Matmuls stay
512 wide (a matmul may not cross a PSUM bank), but sc/av psum tiles are
1024 wide so ACT/DVE amortize their fixed access latency over 2 banks.

Per core, 4 iterations over (pair p, q-range qr of 1024):
  - scores: sc[128 k-rows, 1024 q] per head via block-diag-zeroed rhs copies
    of K^T (bd), lhsT = kt2b k-chunks. exp on ACT -> pts (bf16, unnormalized;
    constant shift 88 is exact for softmax; bf16 keeps fp32 exponent range).
  - attV lags scores by 2 k-chunks so the PE always has ready work while ACT
    paces the pipeline; out^T [65, 1024] accumulates [V|ones]^T @ P-chunk
    per head; row 64 = row-sums.
  - rowsum reciprocal via DVE reciprocal_approx_fast (~5x cheaper than
    reciprocal; 18 bits is plenty), broadcast across partitions with a K=1
    matmul, one DVE multiply, DMA out. The whole normalize tail of iteration
    i is deferred into iteration i+1's score stream (PE FIFO never blocks on
    the DVE chain); only the last iteration pays it inline.

V/K projections are interleaved into iteration 0's score stream in small
bursts so ACT starts exp'ing early and stays the pacing engine throughout.
"""
import numpy as np

import concourse.bacc as bacc
import concourse.mybir as mybir
import concourse.tile as tile
from concourse.bass_utils import run_bass_kernel_spmd

F32 = mybir.dt.float32
F32R = mybir.dt.float32r
BF16 = mybir.dt.bfloat16
EXP = mybir.ActivationFunctionType.Exp

B, N, E, H = 4, 2048, 512, 8
D = E // H            # 64
NCORES = 8
HPC = 4               # heads per core
CSHIFT = 88.0         # exp(S - CSHIFT)
QW = 1024             # q-range width per iteration
NQR = N // QW         # 2
KC = N // 128         # 16 k-chunks
HKC = KC // 2         # k-chunks per pts half-tile

_last_results = None  # stashed BassKernelResults for test.py introspection


def _register_const(nc, val):
    """Extra pre-TileContext f32 [128,1] constant (dep-free, like Bass's
    built-in consts) so activation(bias=val) needs no semaphore wait."""
    t = nc.alloc_sbuf_tensor(f"const-float32-{val}", [128, 1], F32)
    nc.gpsimd.memset(t.ap(), val)
    nc.const_aps.aps[(F32, float(val))] = t.ap()
    nc.all_engine_barrier()


def build_program():
    nc = bacc.Bacc()
    _register_const(nc, -CSHIFT)

    xtb = nc.dram_tensor("xtb", [E, N], BF16, kind="ExternalInput")
    wkt = nc.dram_tensor("wkt", [E, 2 * 128], BF16, kind="ExternalInput")
    wvt = nc.dram_tensor("wvt", [E, 2 * 128], BF16, kind="ExternalInput")
    bk2 = nc.dram_tensor("bk2", [2, 128, 1], F32, kind="ExternalInput")
    bvb = nc.dram_tensor("bvb", [128, 2 * 128], F32, kind="ExternalInput")
    vinit = nc.dram_tensor("vinit", [128, HPC * (D + 1)], BF16, kind="ExternalInput")
    ones64 = nc.dram_tensor("ones64", [33, D], F32R, kind="ExternalInput")
    zrow = nc.dram_tensor("zrow", [1, N], BF16, kind="ExternalInput")
    out_t = nc.dram_tensor("out_t", [HPC, D, N], F32, kind="ExternalOutput")

    with nc.allow_low_precision(reason="bf16 operands are intentional"), \
         tile.TileContext(nc) as tc:
        with (
            tc.tile_pool(name="persist", bufs=1) as per,
            tc.tile_pool(name="work", bufs=2) as work,
            tc.tile_pool(name="mps", bufs=1, space="PSUM") as mps,
        ):
            # ---- persistent SBUF ----
            kt2b = [per.tile([128, N], BF16, name=f"kt2b_{p}") for p in range(2)]
            # block-diagonal rhs copies: bd[0][p] = [KT_even; 0],
            # bd[1][p] = [0; KT_odd]; scores contract K=128 (half zeros) so
            # each head's [128 n, q] block fills all 128 output partitions.
            bd = [[per.tile([128, N], BF16, name=f"bd_{j}_{p}")
                   for p in range(2)] for j in range(2)]
            vs = [per.tile([128, HPC * (D + 1)], BF16, name=f"vs_{t}")
                  for t in range(KC)]
            bvb_sb = per.tile([128, HPC * D], F32)
            bk_sb = [per.tile([128, 1], F32, name=f"bk_{p}") for p in range(2)]
            ones1 = per.tile([33, D], F32R)
            vinit_sb = per.tile([128, HPC * (D + 1)], BF16)

            # ---- input DMA (chunked; sync carries the critical path) ----
            with tc.tile_pool(name="pin", bufs=1) as pin:
                xtb_sb = [pin.tile([128, N], BF16, name=f"xtb_{c}")
                          for c in range(4)]
                wkt_sb = [pin.tile([128, 2 * 128], BF16, name=f"wkt_{c}")
                          for c in range(4)]
                wvt_sb = [pin.tile([128, 2 * 128], BF16, name=f"wvt_{c}")
                          for c in range(4)]
                for p in range(2):
                    nc.sync.dma_start(out=bk_sb[p], in_=bk2[p])
                for c in range(4):
                    nc.sync.dma_start(out=wkt_sb[c],
                                      in_=wkt[128 * c:128 * (c + 1), :])
                for c in range(4):
                    nc.sync.dma_start(out=xtb_sb[c][:, 0:QW],
                                      in_=xtb[128 * c:128 * (c + 1), 0:QW])
                for c in range(4):
                    nc.gpsimd.dma_start(out=wvt_sb[c],
                                        in_=wvt[128 * c:128 * (c + 1), :])
                for c in range(4):
                    nc.gpsimd.dma_start(out=xtb_sb[c][:, QW:N],
                                        in_=xtb[128 * c:128 * (c + 1), QW:N])
                nc.scalar.dma_start(out=bvb_sb, in_=bvb[:])
                nc.scalar.dma_start(out=vinit_sb, in_=vinit[:])
                nc.scalar.dma_start(out=ones1, in_=ones64[:])
                for p in range(2):
                    zb = zrow[:].partition_broadcast(64)
                    nc.scalar.dma_start(out=bd[0][p][64:128, :], in_=zb)
                    nc.scalar.dma_start(out=bd[1][p][0:64, :], in_=zb)
                for t in range(KC):
                    nc.vector.tensor_copy(vs[t][:], vinit_sb[:])

                # ---- emission helpers ----
                def proj_kt2(p, qh):
                    # kt2b[p][:, qh] = (Wk_pair @ x^T + bk_pair), bf16
                    acc = mps.tile([128, QW], F32, tag="sc", bufs=2,
                                   name=f"kacc_{p}_{qh}")
                    for h in range(2):
                        hs = slice(512 * h, 512 * (h + 1))
                        xs = slice(QW * qh + 512 * h, QW * qh + 512 * (h + 1))
                        for c in range(4):
                            nc.tensor.matmul(
                                acc[:, hs],
                                wkt_sb[c][:, 128 * p:128 * (p + 1)],
                                xtb_sb[c][:, xs],
                                start=(c == 0), stop=(c == 3),
                            )
                    qs = slice(QW * qh, QW * (qh + 1))
                    nc.vector.tensor_scalar_add(kt2b[p][:, qs], acc[:],
                                                bk_sb[p][:])
                    nc.vector.tensor_scalar_add(bd[0][p][0:64, qs],
                                                acc[0:64, :], bk_sb[p][0:64])
                    nc.vector.tensor_scalar_add(bd[1][p][64:128, qs],
                                                acc[64:128, :],
                                                bk_sb[p][64:128])

                def proj_v(t):
                    # vs[t] = [V_h0|1|V_h1|1|V_h2|1|V_h3|1] for n-chunk t
                    acc = mps.tile([128, HPC * D], F32, tag="sc", bufs=2,
                                   name=f"vacc_{t}")
                    for c in range(4):
                        nc.tensor.matmul(
                            acc[:],
                            xtb_sb[c][:, 128 * t:128 * (t + 1)],
                            wvt_sb[c][:],
                            start=(c == 0), stop=(c == 3),
                        )
                    vst = vs[t].rearrange("p (h y) -> p h y", h=HPC)
                    nc.vector.tensor_tensor(
                        out=vst[:, :, 0:D],
                        in0=acc[:].rearrange("p (h d) -> p h d", h=HPC),
                        in1=bvb_sb.rearrange("p (h d) -> p h d", h=HPC),
                        op=mybir.AluOpType.add,
                    )

                def pts_at(pts, kc, j):
                    return pts[kc // HKC][j], QW * (kc % HKC)

                def scores_exp(p, qr, pts, kc):
                    q0 = QW * qr
                    for j in range(2):
                        sc = mps.tile([128, QW], F32, tag="sc", bufs=2,
                                      name=f"sc_{p}_{qr}_{kc}_{j}")
                        for h in range(2):
                            hs = slice(512 * h, 512 * (h + 1))
                            nc.tensor.matmul(
                                sc[:, hs],
                                kt2b[p][:, 128 * kc:128 * (kc + 1)],
                                bd[j][p][:, q0 + 512 * h:q0 + 512 * (h + 1)],
                                start=True, stop=True,
                            )
                        pt, col = pts_at(pts, kc, j)
                        nc.scalar.activation(
                            pt[:, col:col + QW],
                            sc[:],
                            EXP, bias=-CSHIFT, scale=1.0,
                        )

                def attv_mm(p, qr, pts, kc, avs):
                    for j in range(2):
                        hl = 2 * p + j
                        if kc == 0:
                            avs.append(mps.tile(
                                [D + 1, QW], F32, tag="av", bufs=2,
                                name=f"av_{p}_{qr}_{j}"))
                        vsl = vs[kc].rearrange("p (h y) -> p h y", h=HPC)
                        pt, col = pts_at(pts, kc, j)
                        for h in range(2):
                            hs = slice(512 * h, 512 * (h + 1))
                            nc.tensor.matmul(
                                avs[j][:, hs],
                                vsl[:, hl, :],
                                pt[:, col + 512 * h:col + 512 * (h + 1)],
                                start=(kc == 0), stop=(kc == KC - 1),
                            )

                def attv_finish(p, qr, avs):
                    # drain av psum -> SBUF, batch both heads' row-sums into
                    # one tile (rows 0 / 32), approx-reciprocal them
                    avsb = []
                    rb = work.tile([33, QW], F32, tag="rb", bufs=2,
                                   name=f"rb_{p}_{qr}")
                    for j in range(2):
                        av_sb = work.tile([D + 1, QW], F32, tag="avsb", bufs=4,
                                          name=f"avsb_{p}_{qr}_{j}")
                        nc.vector.tensor_copy(av_sb[:], avs[j][:])
                        nc.vector.tensor_copy(rb[32 * j:32 * j + 1, :],
                                              av_sb[D:D + 1, :])
                        avsb.append(av_sb)
                    rr32 = work.tile([33, QW], F32, tag="rr32", bufs=2,
                                     name=f"rr32_{p}_{qr}")
                    nc.vector.reciprocal_approx_fast(rr32[:], rb[:])
                    rr = work.tile([33, QW], F32R, tag="rr", bufs=2,
                                   name=f"rr_{p}_{qr}")
                    nc.vector.tensor_copy(rr[:], rr32[:])
                    return (p, qr, avsb, rr)

                def epilogue(state):
                    # normalize + store; deferred into the next iteration's
                    # score stream so the PE FIFO never stalls on the DVE
                    # reciprocal chain
                    p, qr, avsb, rrr = state
                    q0 = QW * qr
                    for j in range(2):
                        hl = 2 * p + j
                        bc = mps.tile([D, QW], F32, tag="sc", bufs=2,
                                      name=f"bc_{p}_{qr}_{j}")
                        for h in range(2):
                            hs = slice(512 * h, 512 * (h + 1))
                            nc.tensor.matmul(
                                bc[:, hs], ones1[32 * j:32 * j + 1, :],
                                rrr[32 * j:32 * j + 1, hs],
                                start=True, stop=True)
                        fin = work.tile([D, QW], F32, tag="fin", bufs=2,
                                        name=f"fin_{p}_{qr}_{j}")
                        nc.vector.tensor_tensor(
                            out=fin[:], in0=avsb[j][0:D, :], in1=bc[:],
                            op=mybir.AluOpType.mult)
                        nc.sync.dma_start(
                            out=out_t[hl, :, q0:q0 + QW], in_=fin[:])

                # ---- flat (iteration, k-chunk) stream with attV lagging by
                # 2 chunks; projections burst into iteration 0; deferred
                # epilogues burst into the middle of the following iteration
                with tc.tile_pool(name="ptp", bufs=1) as ptp:
                    iters = [(p, qr) for p in range(2) for qr in range(NQR)]
                    proj_kt2(0, 0)
                    proj_kt2(0, 1)

                    all_pts = {}
                    all_avs = {}
                    pending = [None]

                    def make_pts_half(it, half):
                        tiles = [
                            ptp.tile([128, HKC * QW], BF16, tag="pt", bufs=6,
                                     name=f"pt_{it}_{half}_{j}")
                            for j in range(2)]
                        if half == 0:
                            all_pts[it] = [tiles]
                            all_avs[it] = []
                        else:
                            all_pts[it].append(tiles)

                    def inject(it, kc):
                        # extra PE work slotted between score chunks
                        if it == 0:
                            if kc == 2:
                                for t in range(0, 8):
                                    proj_v(t)
                            elif kc == 5:
                                for t in range(8, KC):
                                    proj_v(t)
                            elif kc == 8:
                                proj_kt2(1, 0)
                            elif kc == 10:
                                proj_kt2(1, 1)
                        if kc == 6 and pending[0] is not None:
                            epilogue(pending[0])
                            pending[0] = None

                    stream = [(it, kc) for it in range(len(iters))
                              for kc in range(KC)]
                    for idx, (it, kc) in enumerate(stream):
                        p, qr = iters[it]
                        if kc % HKC == 0:
                            make_pts_half(it, kc // HKC)
                        scores_exp(p, qr, all_pts[it], kc)
                        inject(it, kc)
                        if idx >= 2:
                            lit, lkc = stream[idx - 2]
                            lp, lqr = iters[lit]
                            attv_mm(lp, lqr, all_pts[lit], lkc, all_avs[lit])
                            if lkc == KC - 1:
                                pending[0] = attv_finish(lp, lqr,
                                                         all_avs[lit])
                    for idx in (len(stream) - 2, len(stream) - 1):
                        lit, lkc = stream[idx]
                        lp, lqr = iters[lit]
                        attv_mm(lp, lqr, all_pts[lit], lkc, all_avs[lit])
                        if lkc == KC - 1:
                            st = attv_finish(lp, lqr, all_avs[lit])
                            epilogue(st)

    nc.finalize()
    return nc


_program = None


def _vinit():
    v = np.zeros((128, HPC * (D + 1)), dtype=np.float32)
    v[:, D::D + 1] = 1.0
    return v


def kernel(x, Wk, bk, Wv, bv):
    global _program, _last_results
    import ml_dtypes
    bf16 = ml_dtypes.bfloat16
    x = np.asarray(x, dtype=np.float32)
    Wk = np.asarray(Wk, dtype=np.float32)
    bk = np.asarray(bk, dtype=np.float32)
    Wv = np.asarray(Wv, dtype=np.float32)
    bv = np.asarray(bv, dtype=np.float32)

    if _program is None:
        _program = build_program()

    sq = np.float32(1.0 / np.sqrt(E))
    in_maps = []
    for c in range(NCORES):
        b, hg = c // 2, c % 2
        cols = slice(hg * HPC * D, (hg + 1) * HPC * D)
        in_maps.append({
            "xtb": np.ascontiguousarray(x[b].T).astype(bf16),         # [E, N]
            "wkt": np.ascontiguousarray(Wk[cols, :].T).astype(bf16),  # [E, 256]
            "wvt": (np.ascontiguousarray(Wv[cols, :].T) * sq).astype(bf16),
            "bk2": np.ascontiguousarray(bk[cols].reshape(2, 128, 1)),
            "bvb": np.ascontiguousarray(
                np.broadcast_to(bv[cols] * sq, (128, HPC * D))),
            "vinit": _vinit().astype(bf16),
            "ones64": np.ones((33, D), dtype=np.float32),
            "zrow": np.zeros((1, N), dtype=bf16),
        })

    import os
    trace = bool(int(os.environ.get("KERNEL_PROFILE", "0")))
    res = run_bass_kernel_spmd(_program, in_maps, list(range(NCORES)),
                               trace=trace)
    _last_results = res

    out = np.empty((B, N, E), dtype=np.float32)
    for c in range(NCORES):
        b, hg = c // 2, c % 2
        ot = res.results[c]["out_t"]                                 # [4, 64, N]
        for hl in range(HPC):
            out[b, :, hg * HPC * D + hl * D:(hg * HPC * D) + (hl + 1) * D] = \
                ot[hl].T
    return out


# revision 12
# speedup vs baseline: 1.3407x; 1.0553x over previous
"""Trainium2 Bass kernel for BudgetAttentionTwo.

Module: keys = x@Wk.T+bk, values = x@Wv.T+bv (split into 8 heads of 64),
S = K K^T per (b, h), out = (softmax(S)/sqrt(E)) @ V, merged back to [B,N,E].

Sharding: 8 cores, each core owns one batch b = core//2 and four heads
hg*4..hg*4+3 (hg = core%2). No cross-device comms. Weights are pre-sliced,
pre-transposed, and pre-cast to bf16 on the host.

All matmul operands are bf16 (f32 PSUM accumulation; host numpy emulation
puts end-to-end rel err at ~3.3e-3 vs the 2e-2 gate). bf16 buys:
fast-weight-load on 128-wide stationary operands, half the input DMA, and a
pts footprint small enough to double-buffer across iterations (the
baseline's per-iteration pipeline bubble came from pts tiles that could not
overlap, re-throttling the PE's HAM clock every iteration). Matmuls stay
512 wide (a matmul may not cross a PSUM bank), but sc/av psum tiles are
1024 wide so ACT/DVE amortize their fixed access latency over 2 banks.

Per core, 4 iterations over (pair p, q-range qr of 1024):
  - scores: sc[128 k-rows, 1024 q] per head via block-diag-zeroed rhs copies
    of K^T (bd), lhsT = kt2b k-chunks. exp on ACT -> pts (bf16, unnormalized;
    constant shift 88 is exact for softmax; bf16 keeps fp32 exponent range).
  - attV lags scores by 2 k-chunks so the PE always has ready work while ACT
    paces the pipeline; out^T [65, 1024] accumulates [V|ones]^T @ P-chunk
    per head; row 64 = row-sums.
  - rowsum reciprocal via DVE reciprocal_approx_fast (~5x cheaper than
    reciprocal; 18 bits is plenty), broadcast across partitions with a K=1
    matmul, one DVE multiply, DMA out. The whole normalize tail of iteration
    i is deferred into iteration i+1's score stream (PE FIFO never blocks on
    the DVE chain); only the last iteration pays it inline.

V/K projections are interleaved into iteration 0's score stream in small
bursts so ACT starts exp'ing early and stays the pacing engine throughout.
"""
import numpy as np

import concourse.bacc as bacc
import concourse.mybir as mybir
import concourse.tile as tile
from concourse.bass_utils import run_bass_kernel_spmd

F32 = mybir.dt.float32
F32R = mybir.dt.float32r
BF16 = mybir.dt.bfloat16
EXP = mybir.ActivationFunctionType.Exp

B, N, E, H = 4, 2048, 512, 8
D = E // H            # 64
NCORES = 8
HPC = 4               # heads per core
CSHIFT = 88.0         # exp(S - CSHIFT)
QW = 1024             # q-range width per iteration
NQR = N // QW         # 2
KC = N // 128         # 16 k-chunks
HKC = KC // 2         # k-chunks per pts half-tile

_last_results = None  # stashed BassKernelResults for test.py introspection


def _register_const(nc, val):
    """Extra pre-TileContext f32 [128,1] constant (dep-free, like Bass's
    built-in consts) so activation(bias=val) needs no semaphore wait."""
    t = nc.alloc_sbuf_tensor(f"const-float32-{val}", [128, 1], F32)
    nc.gpsimd.memset(t.ap(), val)
    nc.const_aps.aps[(F32, float(val))] = t.ap()
    nc.all_engine_barrier()


def build_program():
    nc = bacc.Bacc()
    _register_const(nc, -CSHIFT)

    xtb = nc.dram_tensor("xtb", [E, N], BF16, kind="ExternalInput")
    wkt = nc.dram_tensor("wkt", [E, 2 * 128], BF16, kind="ExternalInput")
    wvt = nc.dram_tensor("wvt", [E, 2 * 128], BF16, kind="ExternalInput")
    bk2 = nc.dram_tensor("bk2", [2, 128, 1], F32, kind="ExternalInput")
    bvb = nc.dram_tensor("bvb", [128, 2 * 128], F32, kind="ExternalInput")
    vinit = nc.dram_tensor("vinit", [128, HPC * (D + 1)], BF16, kind="ExternalInput")
    ones64 = nc.dram_tensor("ones64", [33, D], F32R, kind="ExternalInput")
    zrow = nc.dram_tensor("zrow", [1, N], BF16, kind="ExternalInput")
    out_t = nc.dram_tensor("out_t", [HPC, D, N], F32, kind="ExternalOutput")

    with nc.allow_low_precision(reason="bf16 operands are intentional"), \
         tile.TileContext(nc) as tc:
        with (
            tc.tile_pool(name="persist", bufs=1) as per,
            tc.tile_pool(name="work", bufs=2) as work,
            tc.tile_pool(name="mps", bufs=1, space="PSUM") as mps,
        ):
            # ---- persistent SBUF ----
            kt2b = [per.tile([128, N], BF16, name=f"kt2b_{p}") for p in range(2)]
            # block-diagonal rhs copies: bd[0][p] = [KT_even; 0],
            # bd[1][p] = [0; KT_odd]; scores contract K=128 (half zeros) so
            # each head's [128 n, q] block fills all 128 output partitions.
            bd = [[per.tile([128, N], BF16, name=f"bd_{j}_{p}")
                   for p in range(2)] for j in range(2)]
            vs = [per.tile([128, HPC * (D + 1)], BF16, name=f"vs_{t}")
                  for t in range(KC)]
            bvb_sb = per.tile([128, HPC * D], F32)
            bk_sb = [per.tile([128, 1], F32, name=f"bk_{p}") for p in range(2)]
            ones1 = per.tile([33, D], F32R)
            vinit_sb = per.tile([128, HPC * (D + 1)], BF16)

            # ---- input DMA (chunked; sync carries the critical path) ----
            with tc.tile_pool(name="pin", bufs=1) as pin:
                xtb_sb = [pin.tile([128, N], BF16, name=f"xtb_{c}")
                          for c in range(4)]
                wkt_sb = [pin.tile([128, 2 * 128], BF16, name=f"wkt_{c}")
                          for c in range(4)]
                wvt_sb = [pin.tile([128, 2 * 128], BF16, name=f"wvt_{c}")
                          for c in range(4)]
                for p in range(2):
                    nc.sync.dma_start(out=bk_sb[p], in_=bk2[p])
                for c in range(4):
                    nc.sync.dma_start(out=wkt_sb[c],
                                      in_=wkt[128 * c:128 * (c + 1), :])
                for c in range(2):
                    nc.sync.dma_start(out=xtb_sb[c][:, 0:QW],
                                      in_=xtb[128 * c:128 * (c + 1), 0:QW])
                for c in range(2, 4):
                    nc.gpsimd.dma_start(out=xtb_sb[c][:, 0:QW],
                                        in_=xtb[128 * c:128 * (c + 1), 0:QW])
                for c in range(4):
                    nc.gpsimd.dma_start(out=xtb_sb[c][:, QW:N],
                                        in_=xtb[128 * c:128 * (c + 1), QW:N])
                for p in range(2):
                    zb = zrow[:].partition_broadcast(64)
                    nc.scalar.dma_start(out=bd[0][p][64:128, :], in_=zb)
                    nc.scalar.dma_start(out=bd[1][p][0:64, :], in_=zb)
                for c in range(4):
                    nc.scalar.dma_start(out=wvt_sb[c],
                                        in_=wvt[128 * c:128 * (c + 1), :])
                nc.scalar.dma_start(out=bvb_sb, in_=bvb[:])
                nc.scalar.dma_start(out=vinit_sb, in_=vinit[:])
                nc.scalar.dma_start(out=ones1, in_=ones64[:])
                for t in range(KC):
                    nc.vector.tensor_copy(vs[t][:], vinit_sb[:])

                # ---- emission helpers ----
                def proj_kt2(p, qh, tag="sc"):
                    # kt2b[p][:, qh] = (Wk_pair @ x^T + bk_pair), bf16.
                    # Injected projections ride the "av" psum tag (attV starts
                    # late in iteration 0) so they never stall the sc pipeline.
                    acc = mps.tile([128, QW], F32, tag=tag, bufs=2,
                                   name=f"kacc_{p}_{qh}")
                    for h in range(2):
                        hs = slice(512 * h, 512 * (h + 1))
                        xs = slice(QW * qh + 512 * h, QW * qh + 512 * (h + 1))
                        for c in range(4):
                            nc.tensor.matmul(
                                acc[:, hs],
                                wkt_sb[c][:, 128 * p:128 * (p + 1)],
                                xtb_sb[c][:, xs],
                                start=(c == 0), stop=(c == 3),
                            )
                    qs = slice(QW * qh, QW * (qh + 1))
                    nc.vector.tensor_scalar_add(kt2b[p][:, qs], acc[:],
                                                bk_sb[p][:])
                    nc.vector.tensor_scalar_add(bd[0][p][0:64, qs],
                                                acc[0:64, :], bk_sb[p][0:64])
                    nc.vector.tensor_scalar_add(bd[1][p][64:128, qs],
                                                acc[64:128, :],
                                                bk_sb[p][64:128])

                def proj_v(t):
                    # vs[t] = [V_h0|1|V_h1|1|V_h2|1|V_h3|1] for n-chunk t
                    acc = mps.tile([128, HPC * D], F32, tag="av", bufs=2,
                                   name=f"vacc_{t}")
                    for c in range(4):
                        nc.tensor.matmul(
                            acc[:],
                            xtb_sb[c][:, 128 * t:128 * (t + 1)],
                            wvt_sb[c][:],
                            start=(c == 0), stop=(c == 3),
                        )
                    vst = vs[t].rearrange("p (h y) -> p h y", h=HPC)
                    nc.vector.tensor_tensor(
                        out=vst[:, :, 0:D],
                        in0=acc[:].rearrange("p (h d) -> p h d", h=HPC),
                        in1=bvb_sb.rearrange("p (h d) -> p h d", h=HPC),
                        op=mybir.AluOpType.add,
                    )

                def pts_at(pts, kc, j):
                    return pts[kc // HKC][j], QW * (kc % HKC)

                def scores_exp(p, qr, pts, kc):
                    q0 = QW * qr
                    for j in range(2):
                        sc = mps.tile([128, QW], F32, tag="sc", bufs=2,
                                      name=f"sc_{p}_{qr}_{kc}_{j}")
                        for h in range(2):
                            hs = slice(512 * h, 512 * (h + 1))
                            nc.tensor.matmul(
                                sc[:, hs],
                                kt2b[p][:, 128 * kc:128 * (kc + 1)],
                                bd[j][p][:, q0 + 512 * h:q0 + 512 * (h + 1)],
                                start=True, stop=True,
                            )
                        pt, col = pts_at(pts, kc, j)
                        nc.scalar.activation(
                            pt[:, col:col + QW],
                            sc[:],
                            EXP, bias=-CSHIFT, scale=1.0,
                        )

                def attv_mm(p, qr, pts, kc, avs):
                    for j in range(2):
                        hl = 2 * p + j
                        if kc == 0:
                            avs.append(mps.tile(
                                [D + 1, QW], F32, tag="av", bufs=2,
                                name=f"av_{p}_{qr}_{j}"))
                        vsl = vs[kc].rearrange("p (h y) -> p h y", h=HPC)
                        pt, col = pts_at(pts, kc, j)
                        for h in range(2):
                            hs = slice(512 * h, 512 * (h + 1))
                            nc.tensor.matmul(
                                avs[j][:, hs],
                                vsl[:, hl, :],
                                pt[:, col + 512 * h:col + 512 * (h + 1)],
                                start=(kc == 0), stop=(kc == KC - 1),
                            )

                def attv_finish(p, qr, avs):
                    # row-sums straight from psum first so the reciprocal
                    # chain (which gates the bc matmuls) completes before the
                    # bulk av drains; both heads' sums share one tile
                    # (rows 0 / 32) so a single approx-reciprocal serves both
                    rb = work.tile([33, QW], F32, tag="rb", bufs=2,
                                   name=f"rb_{p}_{qr}")
                    for j in range(2):
                        nc.vector.tensor_copy(rb[32 * j:32 * j + 1, :],
                                              avs[j][D:D + 1, :])
                    rr32 = work.tile([33, QW], F32, tag="rr32", bufs=2,
                                     name=f"rr32_{p}_{qr}")
                    nc.vector.reciprocal_approx_fast(rr32[:], rb[:])
                    rr = work.tile([33, QW], F32R, tag="rr", bufs=2,
                                   name=f"rr_{p}_{qr}")
                    nc.vector.tensor_copy(rr[:], rr32[:])
                    avsb = []
                    for j in range(2):
                        av_sb = work.tile([D + 1, QW], F32, tag="avsb", bufs=4,
                                          name=f"avsb_{p}_{qr}_{j}")
                        nc.vector.tensor_copy(av_sb[:], avs[j][:])
                        avsb.append(av_sb)
                    return (p, qr, avsb, rr)

                def epilogue(state):
                    # normalize + store; deferred into the next iteration's
                    # score stream so the PE FIFO never stalls on the DVE
                    # reciprocal chain
                    p, qr, avsb, rrr = state
                    q0 = QW * qr
                    for j in range(2):
                        hl = 2 * p + j
                        bc = mps.tile([D, QW], F32, tag="sc", bufs=2,
                                      name=f"bc_{p}_{qr}_{j}")
                        for h in range(2):
                            hs = slice(512 * h, 512 * (h + 1))
                            nc.tensor.matmul(
                                bc[:, hs], ones1[32 * j:32 * j + 1, :],
                                rrr[32 * j:32 * j + 1, hs],
                                start=True, stop=True)
                        fin = work.tile([D, QW], F32, tag="fin", bufs=2,
                                        name=f"fin_{p}_{qr}_{j}")
                        nc.vector.tensor_tensor(
                            out=fin[:], in0=avsb[j][0:D, :], in1=bc[:],
                            op=mybir.AluOpType.mult)
                        nc.sync.dma_start(
                            out=out_t[hl, :, q0:q0 + QW], in_=fin[:])

                # ---- flat (iteration, k-chunk) stream. attV work rides a
                # deque: normally 2 slots behind its exp; the first attV of
                # each iteration defers to slot 4 so the previous iteration's
                # av psum tiles finish draining on the DVE (av tag bufs=2);
                # catch-up is 2 pops/slot. Projections ride the av tag during
                # iteration 0 (attV there starts at kc 12); deferred
                # epilogues fire mid-iteration at kc 6.
                with tc.tile_pool(name="ptp", bufs=1) as ptp:
                    iters = [(p, qr) for p in range(2) for qr in range(NQR)]
                    proj_kt2(0, 0)
                    proj_kt2(0, 1)

                    all_pts = {}
                    all_avs = {}
                    pending = [None]

                    def make_pts_half(it, half):
                        tiles = [
                            ptp.tile([128, HKC * QW], BF16, tag="pt", bufs=6,
                                     name=f"pt_{it}_{half}_{j}")
                            for j in range(2)]
                        if half == 0:
                            all_pts[it] = [tiles]
                            all_avs[it] = []
                        else:
                            all_pts[it].append(tiles)

                    def inject(it, kc):
                        # extra PE work slotted between score chunks, all on
                        # the av psum tag
                        if it == 0:
                            if 2 <= kc <= 5:
                                proj_v(2 * (kc - 2))
                                proj_v(2 * (kc - 2) + 1)
                            elif kc == 6:
                                proj_kt2(1, 0, tag="av")
                            elif kc == 7:
                                proj_kt2(1, 1, tag="av")
                            elif 8 <= kc <= 11:
                                proj_v(2 * (kc - 8) + 8)
                                proj_v(2 * (kc - 8) + 9)
                        if kc == 6 and pending[0] is not None:
                            epilogue(pending[0])
                            pending[0] = None

                    def pop_attv(item):
                        lit, lkc = item
                        lp, lqr = iters[lit]
                        attv_mm(lp, lqr, all_pts[lit], lkc, all_avs[lit])
                        if lkc == KC - 1:
                            pending[0] = attv_finish(lp, lqr, all_avs[lit])

                    stream = [(it, kc) for it in range(len(iters))
                              for kc in range(KC)]
                    todo = []
                    for idx, (it, kc) in enumerate(stream):
                        p, qr = iters[it]
                        if kc % HKC == 0:
                            make_pts_half(it, kc // HKC)
                        scores_exp(p, qr, all_pts[it], kc)
                        inject(it, kc)
                        todo.append((it, kc))
                        pops = 0
                        while todo and pops < 2:
                            lit, lkc = todo[0]
                            if lit * KC + lkc > idx - 2:
                                break  # exp not far enough ahead
                            if lkc == 0 and kc < (12 if lit == 0 else 5):
                                break  # av psum tag not yet free
                            pop_attv(todo.pop(0))
                            pops += 1
                    while todo:
                        pop_attv(todo.pop(0))
                    epilogue(pending[0])
                    pending[0] = None

    nc.finalize()
    return nc


_program = None


def _vinit():
    v = np.zeros((128, HPC * (D + 1)), dtype=np.float32)
    v[:, D::D + 1] = 1.0
    return v


def kernel(x, Wk, bk, Wv, bv):
    global _program, _last_results
    import ml_dtypes
    bf16 = ml_dtypes.bfloat16
    x = np.asarray(x, dtype=np.float32)
    Wk = np.asarray(Wk, dtype=np.float32)
    bk = np.asarray(bk, dtype=np.float32)
    Wv = np.asarray(Wv, dtype=np.float32)
    bv = np.asarray(bv, dtype=np.float32)

    if _program is None:
        _program = build_program()

    sq = np.float32(1.0 / np.sqrt(E))
    in_maps = []
    for c in range(NCORES):
        b, hg = c // 2, c % 2
        cols = slice(hg * HPC * D, (hg + 1) * HPC * D)
        in_maps.append({
            "xtb": np.ascontiguousarray(x[b].T).astype(bf16),         # [E, N]
            "wkt": np.ascontiguousarray(Wk[cols, :].T).astype(bf16),  # [E, 256]
            "wvt": (np.ascontiguousarray(Wv[cols, :].T) * sq).astype(bf16),
            "bk2": np.ascontiguousarray(bk[cols].reshape(2, 128, 1)),
            "bvb": np.ascontiguousarray(
                np.broadcast_to(bv[cols] * sq, (128, HPC * D))),
            "vinit": _vinit().astype(bf16),
            "ones64": np.ones((33, D), dtype=np.float32),
            "zrow": np.zeros((1, N), dtype=bf16),
        })

    import os
    trace = bool(int(os.environ.get("KERNEL_PROFILE", "0")))
    res = run_bass_kernel_spmd(_program, in_maps, list(range(NCORES)),
                               trace=trace)
    _last_results = res

    out = np.empty((B, N, E), dtype=np.float32)
    for c in range(NCORES):
        b, hg = c // 2, c % 2
        ot = res.results[c]["out_t"]                                 # [4, 64, N]
        for hl in range(HPC):
            out[b, :, hg * HPC * D + hl * D:(hg * HPC * D) + (hl + 1) * D] = \
                ot[hl].T
    return out


# revision 16
# speedup vs baseline: 1.4380x; 1.0726x over previous
"""Trainium2 Bass kernel for BudgetAttentionTwo.

Module: keys = x@Wk.T+bk, values = x@Wv.T+bv (split into 8 heads of 64),
S = K K^T per (b, h), out = (softmax(S)/sqrt(E)) @ V, merged back to [B,N,E].

Sharding: 8 cores, each core owns one batch b = core//2 and four heads
hg*4..hg*4+3 (hg = core%2). No cross-device comms. Weights are pre-sliced,
pre-transposed, and pre-cast to bf16 on the host.

All matmul operands are bf16 (f32 PSUM accumulation; host numpy emulation
puts end-to-end rel err at ~3.3e-3 vs the 2e-2 gate). bf16 buys:
fast-weight-load on 128-wide stationary operands, half the input DMA, and a
pts footprint small enough to double-buffer across iterations. Matmuls stay
512 wide (a matmul may not cross a PSUM bank), but sc/av psum tiles are
1024 wide so ACT amortizes its fixed access latency over 2 banks.

The ACT engine (exp over 4 * 2048^2 scores) is the pacing engine: the whole
schedule is built to keep it gapless from ~17us to the end:
  - scores: sc[128 k-rows, 1024 q] per head via block-diag-zeroed rhs copies
    of K^T (bd), lhsT = kt2b k-chunks; exp -> pts (bf16, unnormalized;
    constant shift 88 is exact for softmax; bf16 keeps the fp32 exponent).
  - attV rides a deque lagging the exp stream; the first attV of each
    iteration defers a few chunks so the previous iteration's av psum drains
    on the DVE first (av tag bufs=2). out^T [65, 1024] accumulates
    [V|ones]^T @ P-chunk per head; row 64 = row-sums.
  - rowsum reciprocal via DVE reciprocal_approx_fast (~5x cheaper than
    reciprocal; 18 bits is plenty), broadcast across partitions by the IDLE
    Pool engine (partition_broadcast) -- no psum, no matmul, so the score
    pipeline's psum slots never back up behind the normalize chain. One DVE
    multiply, DMA out. The normalize tail of iteration i is deferred into
    iteration i+1's score stream; only the last iteration pays it inline,
    reading av straight from psum.

K/V projections + vs-tile setup are spread through iteration 0's score
stream on the av psum tag (attV there starts at kc 12), sized so the DVE
never backs up; the very first projection's bias-adds go through ACT so the
first exp isn't serialized behind three DVE adds. Input DMA is chunked
with per-(c, q-half) xtb tiles so the first projection only waits on the
first megabyte.
"""
import numpy as np

import concourse.bacc as bacc
import concourse.mybir as mybir
import concourse.tile as tile
from concourse.bass_utils import run_bass_kernel_spmd

F32 = mybir.dt.float32
BF16 = mybir.dt.bfloat16
EXP = mybir.ActivationFunctionType.Exp

B, N, E, H = 4, 2048, 512, 8
D = E // H            # 64
NCORES = 8
HPC = 4               # heads per core
CSHIFT = 88.0         # exp(S - CSHIFT)
QW = 1024             # q-range width per iteration
NQR = N // QW         # 2
KC = N // 128         # 16 k-chunks
HKC = KC // 2         # k-chunks per pts half-tile

_last_results = None  # stashed BassKernelResults for test.py introspection


def _register_const(nc, val):
    """Extra pre-TileContext f32 [128,1] constant (dep-free, like Bass's
    built-in consts) so activation(bias=val) needs no semaphore wait."""
    t = nc.alloc_sbuf_tensor(f"const-float32-{val}", [128, 1], F32)
    nc.gpsimd.memset(t.ap(), val)
    nc.const_aps.aps[(F32, float(val))] = t.ap()
    nc.all_engine_barrier()


def build_program():
    nc = bacc.Bacc()
    _register_const(nc, -CSHIFT)

    xtb = nc.dram_tensor("xtb", [E, N], BF16, kind="ExternalInput")
    wkt = nc.dram_tensor("wkt", [E, 2 * 128], BF16, kind="ExternalInput")
    wvt = nc.dram_tensor("wvt", [E, 2 * 128], BF16, kind="ExternalInput")
    bk2 = nc.dram_tensor("bk2", [2, 128, 1], F32, kind="ExternalInput")
    bvb = nc.dram_tensor("bvb", [128, 2 * 128], F32, kind="ExternalInput")
    vinit = nc.dram_tensor("vinit", [128, HPC * (D + 1)], BF16, kind="ExternalInput")
    out_t = nc.dram_tensor("out_t", [HPC, D, N], F32, kind="ExternalOutput")

    with nc.allow_low_precision(reason="bf16 operands are intentional"), \
         tile.TileContext(nc) as tc:
        with (
            tc.tile_pool(name="persist", bufs=1) as per,
            tc.tile_pool(name="work", bufs=2) as work,
            tc.tile_pool(name="mps", bufs=1, space="PSUM") as mps,
        ):
            # ---- persistent SBUF ----
            kt2b = [per.tile([128, N], BF16, name=f"kt2b_{p}") for p in range(2)]
            # block-diagonal rhs copies: bd[0][p] = [KT_even; 0],
            # bd[1][p] = [0; KT_odd]; scores contract K=128 (half zeros) so
            # each head's [128 n, q] block fills all 128 output partitions.
            bd = [[per.tile([128, N], BF16, name=f"bd_{j}_{p}")
                   for p in range(2)] for j in range(2)]
            vs = [per.tile([128, HPC * (D + 1)], BF16, name=f"vs_{t}")
                  for t in range(KC)]
            bvb_sb = per.tile([128, HPC * D], F32)
            bk_sb = [per.tile([128, 1], F32, name=f"bk_{p}") for p in range(2)]
            vinit_sb = per.tile([128, HPC * (D + 1)], BF16)

            # pair-0 zero halves now (DVE is idle); pair-1 injected later
            nc.vector.memset(bd[0][0][64:128, :], 0.0)
            nc.vector.memset(bd[1][0][0:64, :], 0.0)

            # ---- input DMA (chunked; sync carries the critical path) ----
            with tc.tile_pool(name="pin", bufs=1) as pin:
                xtb_sb = [[pin.tile([128, QW], BF16, name=f"xtb_{c}_{h}")
                           for h in range(NQR)] for c in range(4)]
                wkt_sb = [pin.tile([128, 2 * 128], BF16, name=f"wkt_{c}")
                          for c in range(4)]
                wvt_sb = [pin.tile([128, 2 * 128], BF16, name=f"wvt_{c}")
                          for c in range(4)]
                for p in range(2):
                    nc.sync.dma_start(out=bk_sb[p], in_=bk2[p])
                for c in range(4):
                    nc.sync.dma_start(out=wkt_sb[c],
                                      in_=wkt[128 * c:128 * (c + 1), :])
                for c in range(2):
                    nc.sync.dma_start(out=xtb_sb[c][0],
                                      in_=xtb[128 * c:128 * (c + 1), 0:QW])
                for c in range(2, 4):
                    nc.gpsimd.dma_start(out=xtb_sb[c][0],
                                        in_=xtb[128 * c:128 * (c + 1), 0:QW])
                for c in range(4):
                    nc.gpsimd.dma_start(out=xtb_sb[c][1],
                                        in_=xtb[128 * c:128 * (c + 1), QW:N])
                for c in range(4):
                    nc.gpsimd.dma_start(out=wvt_sb[c],
                                        in_=wvt[128 * c:128 * (c + 1), :])
                nc.scalar.dma_start(out=bvb_sb, in_=bvb[:])
                nc.scalar.dma_start(out=vinit_sb, in_=vinit[:])

                # ---- emission helpers ----
                def proj_kt2(p, qh, tag="sc", bias_eng=None):
                    # kt2b[p][:, qh] = (Wk_pair @ x^T + bk_pair), bf16.
                    # Injected projections ride the "av" psum tag (attV
                    # starts late in iteration 0) so they never stall the
                    # sc pipeline. bias_eng="act" routes the bd-halves'
                    # bias-adds through ACT (used for the very first
                    # projection, when ACT is idle and DVE is the pacer).
                    acc = mps.tile([128, QW], F32, tag=tag, bufs=2,
                                   name=f"kacc_{p}_{qh}")
                    for h in range(2):
                        hs = slice(512 * h, 512 * (h + 1))
                        for c in range(4):
                            nc.tensor.matmul(
                                acc[:, hs],
                                wkt_sb[c][:, 128 * p:128 * (p + 1)],
                                xtb_sb[c][qh][:, hs],
                                start=(c == 0), stop=(c == 3),
                            )
                    qs = slice(QW * qh, QW * (qh + 1))
                    nc.vector.tensor_scalar_add(kt2b[p][:, qs], acc[:],
                                                bk_sb[p][:])
                    if bias_eng == "act":
                        nc.scalar.add(bd[0][p][0:64, qs], acc[0:64, :],
                                      bk_sb[p][0:64])
                        nc.scalar.add(bd[1][p][64:128, qs], acc[64:128, :],
                                      bk_sb[p][64:128])
                    else:
                        nc.vector.tensor_scalar_add(bd[0][p][0:64, qs],
                                                    acc[0:64, :],
                                                    bk_sb[p][0:64])
                        nc.vector.tensor_scalar_add(bd[1][p][64:128, qs],
                                                    acc[64:128, :],
                                                    bk_sb[p][64:128])

                def proj_v(t):
                    # vs[t] = [V_h0|1|V_h1|1|V_h2|1|V_h3|1] for n-chunk t
                    acc = mps.tile([128, HPC * D], F32, tag="av", bufs=2,
                                   name=f"vacc_{t}")
                    for c in range(4):
                        nc.tensor.matmul(
                            acc[:],
                            xtb_sb[c][t // 8][:, 128 * (t % 8):
                                              128 * (t % 8 + 1)],
                            wvt_sb[c][:],
                            start=(c == 0), stop=(c == 3),
                        )
                    vst = vs[t].rearrange("p (h y) -> p h y", h=HPC)
                    nc.vector.tensor_tensor(
                        out=vst[:, :, 0:D],
                        in0=acc[:].rearrange("p (h d) -> p h d", h=HPC),
                        in1=bvb_sb.rearrange("p (h d) -> p h d", h=HPC),
                        op=mybir.AluOpType.add,
                    )

                def vcopy(t):
                    nc.vector.tensor_copy(vs[t][:], vinit_sb[:])

                def pts_at(pts, kc, j):
                    return pts[kc // HKC][j], QW * (kc % HKC)

                def scores_exp(p, qr, pts, kc):
                    q0 = QW * qr
                    for j in range(2):
                        sc = mps.tile([128, QW], F32, tag="sc", bufs=2,
                                      name=f"sc_{p}_{qr}_{kc}_{j}")
                        for h in range(2):
                            hs = slice(512 * h, 512 * (h + 1))
                            nc.tensor.matmul(
                                sc[:, hs],
                                kt2b[p][:, 128 * kc:128 * (kc + 1)],
                                bd[j][p][:, q0 + 512 * h:q0 + 512 * (h + 1)],
                                start=True, stop=True,
                            )
                        pt, col = pts_at(pts, kc, j)
                        nc.scalar.activation(
                            pt[:, col:col + QW],
                            sc[:],
                            EXP, bias=-CSHIFT, scale=1.0,
                        )

                def attv_mm(p, qr, pts, kc, avs):
                    for j in range(2):
                        hl = 2 * p + j
                        if kc == 0:
                            avs.append(mps.tile(
                                [D + 1, QW], F32, tag="av", bufs=2,
                                name=f"av_{p}_{qr}_{j}"))
                        vsl = vs[kc].rearrange("p (h y) -> p h y", h=HPC)
                        pt, col = pts_at(pts, kc, j)
                        for h in range(2):
                            hs = slice(512 * h, 512 * (h + 1))
                            nc.tensor.matmul(
                                avs[j][:, hs],
                                vsl[:, hl, :],
                                pt[:, col + 512 * h:col + 512 * (h + 1)],
                                start=(kc == 0), stop=(kc == KC - 1),
                            )

                def attv_finish(p, qr, avs, last=False):
                    # row-sums straight from psum first so the reciprocal
                    # chain (which gates the normalize broadcasts) completes
                    # before the bulk av drains
                    rb = work.tile([33, QW], F32, tag="rb", bufs=1,
                                   name=f"rb_{p}_{qr}")
                    for j in range(2):
                        nc.vector.tensor_copy(rb[32 * j:32 * j + 1, :],
                                              avs[j][D:D + 1, :])
                    rr = work.tile([33, QW], F32, tag="rr", bufs=2,
                                   name=f"rr_{p}_{qr}")
                    nc.vector.reciprocal_approx_fast(rr[:], rb[:])
                    # the Pool broadcast only honors a partition-0 source, so
                    # stage head 1's reciprocal row into its own tile
                    rr1 = work.tile([1, QW], F32, tag="rr1", bufs=2,
                                    name=f"rr1_{p}_{qr}")
                    nc.vector.tensor_copy(rr1[:], rr[32:33, :])
                    if last:
                        # tail path: multiply straight out of psum, skip the
                        # SBUF staging copies
                        return (p, qr, avs, (rr, rr1))
                    avsb = []
                    for j in range(2):
                        av_sb = work.tile([D + 1, QW], F32, tag="avsb",
                                          bufs=3, name=f"avsb_{p}_{qr}_{j}")
                        nc.vector.tensor_copy(av_sb[:], avs[j][:])
                        avsb.append(av_sb)
                    return (p, qr, avsb, (rr, rr1))

                def epilogue(state):
                    # normalize + store; the Pool engine broadcasts each
                    # head's 1/rowsum row across 64 partitions (idle engine,
                    # no psum), then one DVE multiply and the output DMA
                    p, qr, avsb, (rr, rr1) = state
                    q0 = QW * qr
                    for j in range(2):
                        hl = 2 * p + j
                        bcst = work.tile([D, QW], F32, tag="bcst", bufs=2,
                                         name=f"bcst_{p}_{qr}_{j}")
                        nc.gpsimd.partition_broadcast(
                            bcst[:], rr[0:1, :] if j == 0 else rr1[0:1, :])
                        fin = work.tile([D, QW], F32, tag="fin", bufs=2,
                                        name=f"fin_{p}_{qr}_{j}")
                        nc.vector.tensor_tensor(
                            out=fin[:], in0=avsb[j][0:D, :], in1=bcst[:],
                            op=mybir.AluOpType.mult)
                        nc.sync.dma_start(
                            out=out_t[hl, :, q0:q0 + QW], in_=fin[:])

                # ---- flat (iteration, k-chunk) stream; see module docstring
                with tc.tile_pool(name="ptp", bufs=1) as ptp:
                    iters = [(p, qr) for p in range(2) for qr in range(NQR)]
                    proj_kt2(0, 0, bias_eng="act")

                    all_pts = {}
                    all_avs = {}
                    pending = [None]

                    def make_pts_half(it, half):
                        tiles = [
                            ptp.tile([128, HKC * QW], BF16, tag="pt", bufs=6,
                                     name=f"pt_{it}_{half}_{j}")
                            for j in range(2)]
                        if half == 0:
                            all_pts[it] = [tiles]
                            all_avs[it] = []
                        else:
                            all_pts[it].append(tiles)

                    # iteration-0 injection schedule (kc -> thunks), sized so
                    # the DVE never backs up more than ~a slot
                    sched0 = {
                        2: [lambda: proj_kt2(0, 1, tag="av")],
                        3: [lambda: [vcopy(t) for t in range(0, 8)]],
                        4: [lambda: proj_v(0), lambda: proj_v(1)],
                        5: [lambda: proj_v(2), lambda: proj_v(3)],
                        6: [lambda: proj_kt2(1, 0, tag="av"),
                            lambda: [vcopy(t) for t in range(8, KC)]],
                        7: [lambda: proj_v(4), lambda: proj_v(5)],
                        8: [lambda: proj_v(6), lambda: proj_v(7)],
                        9: [lambda: proj_kt2(1, 1, tag="av")],
                        10: [lambda: proj_v(8), lambda: proj_v(9),
                             lambda: proj_v(10)],
                        11: [lambda: proj_v(11), lambda: proj_v(12),
                             lambda: proj_v(13)],
                        12: [lambda: proj_v(14), lambda: proj_v(15),
                             lambda: nc.vector.memset(bd[0][1][64:128, :],
                                                      0.0)],
                        13: [lambda: nc.vector.memset(bd[1][1][0:64, :],
                                                      0.0)],
                    }

                    def inject(it, kc):
                        if it == 0:
                            for thunk in sched0.get(kc, ()):
                                thunk()
                        if kc == 6 and pending[0] is not None:
                            epilogue(pending[0])
                            pending[0] = None

                    def pop_attv(item, last_it):
                        lit, lkc = item
                        lp, lqr = iters[lit]
                        attv_mm(lp, lqr, all_pts[lit], lkc, all_avs[lit])
                        if lkc == KC - 1:
                            pending[0] = attv_finish(lp, lqr, all_avs[lit],
                                                     last=last_it)

                    last = len(iters) - 1
                    stream = [(it, kc) for it in range(len(iters))
                              for kc in range(KC)]
                    todo = []
                    for idx, (it, kc) in enumerate(stream):
                        p, qr = iters[it]
                        if kc % HKC == 0:
                            make_pts_half(it, kc // HKC)
                        scores_exp(p, qr, all_pts[it], kc)
                        inject(it, kc)
                        todo.append((it, kc))
                        pops = 0
                        while todo and pops < 2:
                            lit, lkc = todo[0]
                            lag = 1 if lit == last else 2
                            if lit * KC + lkc > idx - lag:
                                break  # exp not far enough ahead
                            # attV of iteration i defers until the previous
                            # iteration's av psum pair has drained (attV of
                            # iteration 0 additionally waits for the
                            # projections, which borrow the av tag)
                            if lkc == 0 and it == lit and \
                                    kc < (13 if lit == 0 else 5):
                                break
                            pop_attv(todo.pop(0), lit == last)
                            pops += 1
                    while todo:
                        pop_attv(todo.pop(0), True)
                    epilogue(pending[0])
                    pending[0] = None

    nc.finalize()
    return nc


_program = None


def _vinit():
    v = np.zeros((128, HPC * (D + 1)), dtype=np.float32)
    v[:, D::D + 1] = 1.0
    return v


def kernel(x, Wk, bk, Wv, bv):
    global _program, _last_results
    import ml_dtypes
    bf16 = ml_dtypes.bfloat16
    x = np.asarray(x, dtype=np.float32)
    Wk = np.asarray(Wk, dtype=np.float32)
    bk = np.asarray(bk, dtype=np.float32)
    Wv = np.asarray(Wv, dtype=np.float32)
    bv = np.asarray(bv, dtype=np.float32)

    if _program is None:
        _program = build_program()

    sq = np.float32(1.0 / np.sqrt(E))
    in_maps = []
    for c in range(NCORES):
        b, hg = c // 2, c % 2
        cols = slice(hg * HPC * D, (hg + 1) * HPC * D)
        in_maps.append({
            "xtb": np.ascontiguousarray(x[b].T).astype(bf16),         # [E, N]
            "wkt": np.ascontiguousarray(Wk[cols, :].T).astype(bf16),  # [E, 256]
            "wvt": (np.ascontiguousarray(Wv[cols, :].T) * sq).astype(bf16),
            "bk2": np.ascontiguousarray(bk[cols].reshape(2, 128, 1)),
            "bvb": np.ascontiguousarray(
                np.broadcast_to(bv[cols] * sq, (128, HPC * D))),
            "vinit": _vinit().astype(bf16),
        })

    import os
    trace = bool(int(os.environ.get("KERNEL_PROFILE", "0")))
    res = run_bass_kernel_spmd(_program, in_maps, list(range(NCORES)),
                               trace=trace)
    _last_results = res

    out = np.empty((B, N, E), dtype=np.float32)
    for c in range(NCORES):
        b, hg = c // 2, c % 2
        ot = res.results[c]["out_t"]                                 # [4, 64, N]
        for hl in range(HPC):
            out[b, :, hg * HPC * D + hl * D:(hg * HPC * D) + (hl + 1) * D] = \
                ot[hl].T
    return out
